# revision 1
# baseline (speedup 1.0000x reference)
"""Sparse transposed-conv (27-tap gather-GEMM) + BatchNorm + LeakyReLU on 8 TRN2 cores.

Strategy (voxel-sharded, compacted kernel map):
  - Host compacts nbr into per-(core, tap, src-window) valid (src, dst) pair lists
    (~87% of nbr entries are -1 and are dropped entirely).
  - Device, per core: dma_gather valid x rows (int16 idx, windowed source);
    PE-transpose 128x128 chunks into channel-major; one fp32 matmul per 512
    columns against a block-diag [[Wk,0],[0,Wk]] stationary (2 voxels per
    streamed column); PE-transpose back; dma_scatter_add into an HBM h
    accumulator (per-tap, destinations unique within each scatter).
  - Center tap (identity map) runs as a dense pipeline and initializes h.
  - BN tail: stream h back transposed, per-channel sum/sumsq, 8-core AllReduce,
    fused Lrelu(h*scale+bias) on ACT, transpose back, write y.
"""
import os
import numpy as np

import concourse.bass as bass
import concourse.mybir as mybir
import concourse.bacc as bacc
import concourse.tile as tile
from concourse import bass_utils
from concourse.masks import make_identity

N = 200000
C = 8
V = N // C          # 25000 voxels per core
D = 64
K = 27
KC = 13             # center tap (identity map)
VP = 25600          # h rows incl. trash zone
TRASH = 25000
WIN = 32768
NW = (N + WIN - 1) // WIN   # 7 source windows
EPS = 1e-5
NEG = 0.01
F32 = mybir.dt.float32
I16 = mybir.dt.int16


def _r128(n):
    return (n + 127) & ~127


def _r256(n):
    return (n + 255) & ~255


def _pack16(slab, col0, vals):
    """Place index list (len mult of 16) at int16-slab columns col0.., wrapped
    [i%16, i//16] and replicated to all 8 Q7 core partition groups."""
    w = vals.reshape(-1, 16).T  # [16, L/16]
    L16 = w.shape[1]
    for r in range(8):
        slab[r * 16:(r + 1) * 16, col0:col0 + L16] = w
    return col0 + L16


def _prep_host(nbr):
    """Compact kernel maps. Returns per-k segment plan + per-core idx slabs."""
    ks = [k for k in range(K) if k != KC]
    # per (c, k, w): rel-src list + dst list
    lists = {}
    for c in range(C):
        sl = slice(c * V, (c + 1) * V)
        for k in ks:
            src = nbr[k, sl]
            valid = np.nonzero(src >= 0)[0]
            s = src[valid]
            w_of = s // WIN
            for w in range(NW):
                m = w_of == w
                lists[(c, k, w)] = (
                    (s[m] - w * WIN).astype(np.int16),
                    valid[m].astype(np.int16),
                )
    seg_len = {}
    for k in ks:
        for w in range(NW):
            mx = max(len(lists[(c, k, w)][0]) for c in range(C))
            seg_len[(k, w)] = _r128(mx)
    NK = {}
    for k in ks:
        tot = sum(seg_len[(k, w)] for w in range(NW))
        tot2 = _r256(tot)
        if tot2 > tot:
            # fold the round-up into the last nonempty window so every
            # position is covered by a gather
            for w in reversed(range(NW)):
                if seg_len[(k, w)] > 0:
                    seg_len[(k, w)] += tot2 - tot
                    break
            else:
                seg_len[(k, NW - 1)] += tot2 - tot
        NK[k] = tot2
    GT = sum(NK[k] for k in ks)          # total gather/scatter positions
    gslab = np.zeros((C, 128, GT // 16), np.int16)
    sslab = np.zeros((C, 128, GT // 16), np.int16)
    plan = []   # (k, [(w, seg_len)...], koff) in emission order
    koff = 0
    for k in ks:
        segs = [(w, seg_len[(k, w)]) for w in range(NW) if seg_len[(k, w)] > 0]
        plan.append((k, segs, koff))
        for c in range(C):
            col = koff // 16
            sc = []
            for w, L in segs:
                g, s = lists[(c, k, w)]
                gp = np.full(L, 0, np.int16)
                gp[:len(g)] = g
                sp = np.full(L, TRASH, np.int16)
                sp[:len(s)] = s
                col = _pack16(gslab[c], col, gp)
                sc.append(sp)
            _pack16(sslab[c], koff // 16, np.concatenate(sc))
        koff += NK[k]
    return plan, NK, GT, gslab, sslab


KPHASE = int(os.environ.get("KPHASE", "4")) if os.environ.get("KERNEL_DEBUG") else 4


def _early_y(nc, y_d, h_d, io):  # h_d = h_a
    yv0 = y_d[:, :].rearrange("(g p m) c -> g p (m c)", p=125, m=8)
    hv0 = h_d[0:25000, :].rearrange("(g p m) c -> g p (m c)", p=125, m=8)
    for g in range(25):
        hb0 = io.tile([128, 512], F32, tag="hb0")
        nc.sync.dma_start(hb0[0:125, :], hv0[g])
        nc.sync.dma_start(yv0[g], hb0[0:125, :])


class _EarlyExitException(Exception):
    pass


def _build_program(plan, GT, NKmax):
    nc = bacc.Bacc("TRN2", target_bir_lowering=False, debug=False, num_devices=C)

    try:
        _build_body(nc)
    except _EarlyExitException:
        pass
    nc.compile()
    return nc


def _build_body(nc):
    plan, GT, NKmax = _CURRENT_PLAN
    x_d = nc.dram_tensor("x_d", [N, D], F32, kind="ExternalInput")
    xc_d = nc.dram_tensor("xc_d", [V, D], F32, kind="ExternalInput")
    W_d = nc.dram_tensor("W_d", [K, D, D], F32, kind="ExternalInput")
    gam_d = nc.dram_tensor("gam_d", [1, D], F32, kind="ExternalInput")
    bet_d = nc.dram_tensor("bet_d", [1, D], F32, kind="ExternalInput")
    gi_d = nc.dram_tensor("gi_d", [128, GT // 16], I16, kind="ExternalInput")
    si_d = nc.dram_tensor("si_d", [128, GT // 16], I16, kind="ExternalInput")
    y_d = nc.dram_tensor("y_d", [V, D], F32, kind="ExternalOutput")

    with tile.TileContext(nc) as tc:
        with tc.tile_pool(name="sb", bufs=1) as sb, \
             tc.tile_pool(name="io", bufs=3) as io, \
             tc.tile_pool(name="ps", bufs=2, space="PSUM") as ps, \
             tc.tile_pool(name="dram", bufs=1, space="DRAM") as dram:

            h_a = dram.tile([VP, D], F32)
            h_b = dram.tile([VP, D], F32)

            ident = sb.tile([128, 128], F32)
            make_identity(nc, ident[:])
            gi_t = sb.tile([128, GT // 16], I16)
            nc.sync.dma_start(gi_t[:], gi_d[:, :])
            si_t = sb.tile([128, GT // 16], I16)
            nc.sync.dma_start(si_t[:], si_d[:, :])

            W2 = sb.tile([128, K * 128], F32)
            nc.gpsimd.memset(W2[:], 0.0)
            for k in range(K):
                nc.sync.dma_start(W2[0:D, k * 128:k * 128 + D], W_d[k, :, :])
                nc.sync.dma_start(W2[D:128, k * 128 + D:(k + 1) * 128], W_d[k, :, :])

            zt = sb.tile([128, 512], F32)
            nc.gpsimd.memset(zt[:], 0.0)

            # trash-zone zero init: rows [24576, 25600) (center overwrites
            # the 24576..24999 overlap afterwards)
            nc.sync.dma_start(
                h_a[24576:VP, :].rearrange("(p m) c -> p (m c)", p=128), zt[:])
            hbz = h_b[:, :].rearrange("(p m) c -> p (m c)", p=128)
            for g in range(25):
                nc.sync.dma_start(hbz[:, g * 512:(g + 1) * 512], zt[:])

            def mid(gsrc, kk, cols, sout, scol):
                """gsrc[:, cols] (2-slot chunks) -> transpose -> MM W2[kk]
                -> transpose back -> sout[:, scol:scol+len(cols)]."""
                gw = len(cols) * 128
                pa = ps.tile([128, 512], F32, tag="psA", space="PSUM")
                for j, cj in enumerate(cols):
                    nc.tensor.transpose(
                        out=pa[:, j * 128:(j + 1) * 128],
                        in_=gsrc[:, cj * 128:(cj + 1) * 128], identity=ident[:])
                ct = io.tile([128, 512], F32, tag="ct")
                nc.vector.tensor_copy(ct[:, :gw], pa[:, :gw])
                pb = ps.tile([128, 512], F32, tag="psB", space="PSUM")
                nc.tensor.matmul(out=pb[:, :gw],
                                 lhsT=W2[:, kk * 128:(kk + 1) * 128],
                                 rhs=ct[:, :gw], start=True, stop=True)
                hb = io.tile([128, 512], F32, tag="hb")
                nc.vector.tensor_copy(hb[:, :gw], pb[:, :gw])
                pc = ps.tile([128, 512], F32, tag="psC", space="PSUM")
                for j in range(len(cols)):
                    nc.tensor.transpose(
                        out=pc[:, j * 128:(j + 1) * 128],
                        in_=hb[:, j * 128:(j + 1) * 128], identity=ident[:])
                nc.scalar.activation(sout[:, scol:scol + gw], pc[:, :gw],
                                     mybir.ActivationFunctionType.Copy, bias=0.0)

            # ---- center tap: dense, initializes h rows 0..24999 ----
            xcv = xc_d[:, :].rearrange("(g p m) c -> g p (m c)", p=125, m=8)
            hv = h_a[0:V, :].rearrange("(g p m) c -> g p (m c)", p=125, m=8)
            for g in range(25):
                xg = io.tile([128, 512], F32, tag="xg")
                nc.sync.dma_start(xg[0:125, :], xcv[g])
                so = io.tile([128, 512], F32, tag="dso")
                mid(xg, KC, [0, 1, 2, 3], so, 0)
                nc.sync.dma_start(hv[g], so[0:125, :])

            # ---- 26 sparse taps ----
            for ki, (k, segs, koff) in enumerate(plan if KPHASE >= 2 else []):
                h_t = h_a if ki % 2 == 0 else h_b
                NKk = sum(L for _, L in segs)
                gb = io.tile([128, NKmax // 128, D], F32, tag="gb")
                soff = 0
                for w, L in segs:
                    wlo = w * WIN
                    whi = min(N, wlo + WIN)
                    nc.gpsimd.dma_gather(
                        out_ap=gb[:, soff // 128:(soff + L) // 128, :],
                        in_ap=x_d[wlo:whi, :],
                        idxs_ap=gi_t[:, (koff + soff) // 16:(koff + soff + L) // 16],
                        num_idxs=L, num_idxs_reg=L, elem_size=D,
                        single_packet=False)
                    soff += L
                gbf = gb[:].rearrange("p m d -> p (m d)")
                sk = io.tile([128, NKmax // 128, D], F32, tag="sk")
                skf = sk[:].rearrange("p m d -> p (m d)")
                nch = NKk // 256
                for c0 in range(0, nch, 4):
                    cols = list(range(c0, min(c0 + 4, nch)))
                    mid(gbf, k, cols, skf, c0 * 128)
                nc.gpsimd.dma_scatter_add(
                    out_ap=h_t[:, :], in_ap=sk[:, 0:NKk // 128, :],
                    idxs_ap=si_t[:, koff // 16:(koff + NKk) // 16],
                    num_idxs=NKk, num_idxs_reg=NKk, elem_size=D,
                    single_packet=False)

            if KPHASE < 3:
                yv0 = y_d[:, :].rearrange("(g p m) c -> g p (m c)", p=125, m=8)
                hv0 = h_d[0:V, :].rearrange("(g p m) c -> g p (m c)", p=125, m=8)
                for g in range(25):
                    hb0 = io.tile([128, 512], F32, tag="hb0")
                    nc.sync.dma_start(hb0[0:125, :], hv0[g])
                    nc.sync.dma_start(yv0[g], hb0[0:125, :])
                return
            # zero the trash zones (rows 25000..25599) so stats see zeros there
            nc.sync.dma_start(
                h_a[TRASH:VP, :].rearrange("(p m) c -> p (m c)", p=75),
                zt[0:75, :])
            nc.sync.dma_start(
                h_b[TRASH:VP, :].rearrange("(p m) c -> p (m c)", p=75),
                zt[0:75, :])

            # ---- BN stats: stream h transposed, keep hT resident ----
            KSTAT = int(os.environ.get("KSTAT", "4")) if os.environ.get("KERNEL_DEBUG") else 4
            hpv = h_a[:, :].rearrange("(p m) c -> p (m c)", p=128)  # [128, 12800]
            hpvb = h_b[:, :].rearrange("(p m) c -> p (m c)", p=128)
            hT = sb.tile([128, 12800], F32)
            sacc = sb.tile([128, 32], F32)
            qacc = sb.tile([128, 32], F32)
            nc.gpsimd.memset(sacc[:], 0.0)
            nc.gpsimd.memset(qacc[:], 0.0)
            for g in range(25):
                hi = io.tile([128, 512], F32, tag="hi")
                nc.sync.dma_start(hi[:], hpv[:, g * 512:(g + 1) * 512])
                hib = io.tile([128, 512], F32, tag="hib")
                nc.sync.dma_start(hib[:], hpvb[:, g * 512:(g + 1) * 512])
                nc.vector.tensor_tensor(out=hi[:], in0=hi[:], in1=hib[:],
                                        op=mybir.AluOpType.add)
                pa = ps.tile([128, 512], F32, tag="psA", space="PSUM")
                for j in range(4):
                    nc.tensor.transpose(out=pa[:, j * 128:(j + 1) * 128],
                                        in_=hi[:, j * 128:(j + 1) * 128],
                                        identity=ident[:])
                nc.vector.tensor_copy(hT[:, g * 512:(g + 1) * 512], pa[:])
            if KSTAT < 2:
                _early_y(nc, y_d, h_d, io)
                return
            for g in range(25):
                ch = hT[:, g * 512:(g + 1) * 512]
                nc.vector.tensor_reduce(out=sacc[:, g:g + 1], in_=ch,
                                        axis=mybir.AxisListType.X,
                                        op=mybir.AluOpType.add)
                scr = io.tile([128, 512], F32, tag="scr")
                nc.vector.tensor_tensor(out=scr[:], in0=ch, in1=ch,
                                        op=mybir.AluOpType.mult)
                nc.vector.tensor_reduce(out=qacc[:, g:g + 1], in_=scr[:],
                                        axis=mybir.AxisListType.X,
                                        op=mybir.AluOpType.add)
            sq = sb.tile([128, 2], F32)
            nc.vector.tensor_reduce(out=sq[:, 0:1], in_=sacc[:, 0:25],
                                    axis=mybir.AxisListType.X,
                                    op=mybir.AluOpType.add)
            nc.vector.tensor_reduce(out=sq[:, 1:2], in_=qacc[:, 0:25],
                                    axis=mybir.AxisListType.X,
                                    op=mybir.AluOpType.add)
            # fold channel parity halves (partitions p and p+64) via an
            # SBUF->SBUF DMA re-partition, then all-reduce over cores
            sqh = sb.tile([64, 2], F32)
            nc.sync.dma_start(sqh[:], sq[64:128, :])
            sq64 = sb.tile([64, 2], F32)
            nc.vector.tensor_tensor(out=sq64[:], in0=sq[0:64, :], in1=sqh[:],
                                    op=mybir.AluOpType.add)
            if KSTAT < 3:
                _early_y(nc, y_d, h_d, io)
                return
            cc_in = dram.tile([64, 2], F32)
            cc_out = dram.tile([64, 2], F32)
            nc.gpsimd.dma_start(cc_in[:], sq64[:])
            if KPHASE >= 4:
                nc.gpsimd.collective_compute(
                    "AllReduce", mybir.AluOpType.add,
                    replica_groups=[list(range(C))],
                    ins=[cc_in.opt()], outs=[cc_out.opt()])
            else:
                nc.gpsimd.dma_start(cc_out[:], cc_in[:])
            g2 = sb.tile([64, 2], F32)
            nc.sync.dma_start(g2[:], cc_out[:])
            # per-channel (partition-major) BN coefficients
            me = sb.tile([64, 2], F32)
            nc.vector.tensor_scalar_mul(me[:], g2[:], 1.0 / N)  # [mean, Eh2]
            v1 = sb.tile([64, 1], F32)
            nc.vector.tensor_tensor(out=v1[:], in0=me[:, 0:1], in1=me[:, 0:1],
                                    op=mybir.AluOpType.mult)
            nc.vector.tensor_tensor(out=v1[:], in0=me[:, 1:2], in1=v1[:],
                                    op=mybir.AluOpType.subtract)
            eps_t = sb.tile([64, 1], F32)
            nc.gpsimd.memset(eps_t[:], EPS)
            std = sb.tile([64, 1], F32)
            nc.scalar.activation(std[:], v1[:], mybir.ActivationFunctionType.Sqrt,
                                 bias=eps_t[:])
            rin = sb.tile([64, 1], F32)
            nc.vector.reciprocal(rin[:], std[:])
            gam = sb.tile([64, 1], F32)
            nc.sync.dma_start(gam[:], gam_d[0, :, None])
            bet = sb.tile([64, 1], F32)
            nc.sync.dma_start(bet[:], bet_d[0, :, None])
            sc_h = sb.tile([64, 1], F32)
            nc.vector.tensor_tensor(out=sc_h[:], in0=rin[:], in1=gam[:],
                                    op=mybir.AluOpType.mult)
            cb_h = sb.tile([64, 1], F32)
            nc.vector.tensor_tensor(out=cb_h[:], in0=me[:, 0:1], in1=sc_h[:],
                                    op=mybir.AluOpType.mult)
            nc.vector.tensor_tensor(out=cb_h[:], in0=bet[:], in1=cb_h[:],
                                    op=mybir.AluOpType.subtract)
            s128 = sb.tile([128, 2], F32)
            nc.sync.dma_start(s128[0:64, 0:1], sc_h[:])
            nc.sync.dma_start(s128[64:128, 0:1], sc_h[:])
            nc.sync.dma_start(s128[0:64, 1:2], cb_h[:])
            nc.sync.dma_start(s128[64:128, 1:2], cb_h[:])

            if KSTAT < 4:
                _early_y(nc, y_d, h_d, io)
                return
            # ---- apply Lrelu(h*s + c) in hT space, transpose back, write y ----
            yv = y_d[:, :].rearrange("(p m) c -> p (m c)", p=125)  # [125, 12800]
            for g in range(25):
                ha = io.tile([128, 512], F32, tag="ha")
                nc.scalar.activation(ha[:], hT[:, g * 512:(g + 1) * 512],
                                     mybir.ActivationFunctionType.Lrelu,
                                     bias=s128[:, 1:2], scale=s128[:, 0:1],
                                     alpha=NEG)
                pc = ps.tile([128, 512], F32, tag="psC", space="PSUM")
                for j in range(4):
                    nc.tensor.transpose(out=pc[:, j * 128:(j + 1) * 128],
                                        in_=ha[:, j * 128:(j + 1) * 128],
                                        identity=ident[:])
                yo = io.tile([128, 512], F32, tag="yo")
                nc.vector.tensor_copy(yo[:], pc[:])
                nc.sync.dma_start(yv[:, g * 512:(g + 1) * 512], yo[0:125, :])

_CACHE = {}
_CURRENT_PLAN = None


def build(nbr):
    key = nbr.tobytes()[:4096] + nbr.tobytes()[-4096:]
    if key in _CACHE:
        return _CACHE[key]
    plan, NK, GT, gslab, sslab = _prep_host(np.asarray(nbr, np.int64))
    NKmax = max(NK.values())
    global _CURRENT_PLAN
    _CURRENT_PLAN = (plan, GT, NKmax)
    nc = _build_program(plan, GT, NKmax)
    _CACHE[key] = (nc, gslab, sslab)
    return nc, gslab, sslab


def kernel(x, W, gamma, beta, nbr):
    x = np.ascontiguousarray(np.asarray(x, np.float32))
    W = np.ascontiguousarray(np.asarray(W, np.float32))
    gamma = np.asarray(gamma, np.float32).reshape(1, D)
    beta = np.asarray(beta, np.float32).reshape(1, D)
    nbr = np.asarray(nbr)
    nc, gslab, sslab = build(nbr)
    in_maps = []
    for c in range(C):
        in_maps.append({
            "x_d": x,
            "xc_d": x[c * V:(c + 1) * V],
            "W_d": W,
            "gam_d": gamma,
            "bet_d": beta,
            "gi_d": gslab[c],
            "si_d": sslab[c],
        })
    res = bass_utils.run_bass_kernel_spmd(nc, in_maps, core_ids=list(range(C)))
    return np.concatenate([res.results[c]["y_d"] for c in range(C)], axis=0)



# revision 10
# speedup vs baseline: 5.1927x; 5.1927x over previous
"""Sparse transposed-conv (27-tap gather-GEMM) + BatchNorm + LeakyReLU on 8 TRN2 cores.

Strategy (component-sharded, SBUF-resident accumulator):
  - Host: connected components of the neighbor graph are contiguous in a
    global reorder; each core owns 25000 voxels plus a small halo of
    cross-boundary sources, so every gather is core-local (< 32768 rows,
    single int16 window -> one dma_gather per tap).
  - Device, per core: per tap dma_gather x rows -> PE-transpose 128x128
    chunks to channel-major -> one fp32 matmul per 512 columns against a
    block-diag [[Wk,0],[0,Wk]] stationary -> PE-transpose back ->
    dma_scatter_add into an SBUF-resident h accumulator (parity-split
    CCE add; v -> parity (v>>7)&1, partition v%128, group v>>8).
  - Center tap runs dense and writes h with strided copies (no scatter).
  - BN: per-channel sum/sumsq reduced on DVE directly from SBUF h,
    partition-summed with a ones-matmul, 8-core AllReduce of [64,2],
    then y = lrelu(h*s + b) applied in place and DMA'd out.
"""
import numpy as np

import concourse.bass as bass
import concourse.mybir as mybir
import concourse.bacc as bacc
import concourse.tile as tile
from concourse import bass_utils
from concourse.masks import make_identity

N = 200000
C = 8
V = N // C          # 25000 voxels per core
D = 64
K = 27
KC = 13             # center tap (identity map)
GRP = 100           # h groups per parity buffer (covers v < 25600)
NSTAT = 98          # groups included in BN stats / y (v < 25088)
TR0 = 25088         # trash dst base (groups 98, 99)
EPS = 1e-5
NEG = 0.01
F32 = mybir.dt.float32
I16 = mybir.dt.int16


def _r128(n):
    return (n + 127) & ~127


def _r256(n):
    return (n + 255) & ~255


def _pack16(slab, col0, vals):
    """Place index list (len mult of 16) at int16-slab columns col0.., wrapped
    [i%16, i//16] and replicated to all 8 Q7 core partition groups."""
    w = vals.reshape(-1, 16).T
    L16 = w.shape[1]
    for r in range(8):
        slab[r * 16:(r + 1) * 16, col0:col0 + L16] = w
    return col0 + L16


def _components(nbr):
    import scipy.sparse as sp
    import scipy.sparse.csgraph as csg
    Kk, n = nbr.shape
    src = np.repeat(np.arange(n), Kk)
    dst = nbr.T.ravel()
    m = dst >= 0
    A = sp.coo_matrix((np.ones(m.sum(), np.int8), (src[m], dst[m])),
                      shape=(n, n))
    _, lab = csg.connected_components(A, directed=False)
    return lab


def _prep_host(nbr):
    """Component-contiguous reorder + per-core compacted local kernel maps."""
    nbr = np.asarray(nbr, np.int64)
    lab = _components(nbr)
    # balance whole components across cores (largest-first, least-loaded)
    import heapq
    sizes = np.bincount(lab)
    assign = np.empty(len(sizes), np.int64)
    heap = [(0, c) for c in range(C)]
    heapq.heapify(heap)
    for comp in np.argsort(sizes)[::-1]:
        load, c = heapq.heappop(heap)
        assign[comp] = c
        heapq.heappush(heap, (load + int(sizes[comp]), c))
    perm = np.lexsort((lab, assign[lab]))      # rank -> orig
    rank = np.empty(N, np.int64)
    rank[perm] = np.arange(N)                  # orig -> rank

    # per-(core, tap) local (src, dst) pair lists
    ks = [k for k in range(K) if k != KC]
    nbr_r = np.where(nbr >= 0, rank[np.clip(nbr, 0, None)], -1)  # in rank space
    nbr_r = nbr_r[:, perm]                     # column r: dst rank r
    lists = {}
    halos = []
    for c in range(C):
        lo, hi = c * V, (c + 1) * V
        ext = set()
        per_k = {}
        for k in ks:
            srcs = nbr_r[k, lo:hi]
            valid = np.nonzero(srcs >= 0)[0]
            s = srcs[valid]
            per_k[k] = (s, valid)
            out = s[(s < lo) | (s >= hi)]
            ext.update(out.tolist())
        halo = np.sort(np.fromiter(ext, np.int64, len(ext)))
        halos.append(halo)
        for k in ks:
            s, valid = per_k[k]
            inr = (s >= lo) & (s < hi)
            loc = np.where(inr, s - lo, V + np.searchsorted(halo, s))
            o = np.argsort(loc, kind="stable")
            lists[(c, k)] = (loc[o].astype(np.int16), valid[o].astype(np.int16))

    XL = _r128(V + max(len(h) for h in halos) + 1)
    assert XL <= 32768, XL

    NK = {k: max(256, _r256(max(len(lists[(c, k)][0]) for c in range(C))))
          for k in ks}
    GT = sum(NK.values())
    max_npad = max(NK[k] - len(lists[(c, k)][0])
                   for k in ks for c in range(C))
    grp = max(GRP, -(-(TR0 + max_npad) // 256))
    gslab = np.zeros((C, 128, GT // 16), np.int16)
    sslab = np.zeros((C, 128, GT // 16), np.int16)
    plan = []
    koff = 0
    for k in ks:
        plan.append((k, NK[k], koff))
        for c in range(C):
            g, s = lists[(c, k)]
            L = NK[k]
            gp = np.zeros(L, np.int16)
            gp[:len(g)] = g
            sp_ = np.empty(L, np.int16)
            sp_[:len(s)] = s
            npad = L - len(s)
            assert npad <= 256 * grp - TR0, (npad, grp)
            if npad:
                sp_[len(s):] = TR0 + np.arange(npad, dtype=np.int16)
            _pack16(gslab[c], koff // 16, gp)
            _pack16(sslab[c], koff // 16, sp_)
        koff += NK[k]

    xsel = []
    for c in range(C):
        sel = np.concatenate([perm[c * V:(c + 1) * V], perm[halos[c]]])
        xsel.append(sel)
    return plan, GT, XL, grp, gslab, sslab, xsel, perm


def _build_program(plan, GT, XL, grp):
    nc = bacc.Bacc("TRN2", target_bir_lowering=False, debug=False,
                   num_devices=C, num_swdge_queues=4)

    xc_d = nc.dram_tensor("xc_d", [XL, D], F32, kind="ExternalInput")
    W_d = nc.dram_tensor("W_d", [K, D, D], F32, kind="ExternalInput")
    gam_d = nc.dram_tensor("gam_d", [1, D], F32, kind="ExternalInput")
    bet_d = nc.dram_tensor("bet_d", [1, D], F32, kind="ExternalInput")
    gi_d = nc.dram_tensor("gi_d", [128, GT // 16], I16, kind="ExternalInput")
    si_d = nc.dram_tensor("si_d", [128, GT // 16], I16, kind="ExternalInput")
    y_d = nc.dram_tensor("y_d", [V, D], F32, kind="ExternalOutput")

    NKmax = max(nk for _, nk, _ in plan)

    with tile.TileContext(nc) as tc:
        with tc.tile_pool(name="sb", bufs=1) as sb, \
             tc.tile_pool(name="io", bufs=3) as io, \
             tc.tile_pool(name="ps", bufs=2, space="PSUM") as ps, \
             tc.tile_pool(name="dram", bufs=1, space="DRAM") as dram:

            ident = sb.tile([128, 128], F32)
            make_identity(nc, ident[:])
            gi_t = sb.tile([128, GT // 16], I16)
            nc.sync.dma_start(gi_t[:], gi_d[:, :])
            si_t = sb.tile([128, GT // 16], I16)
            nc.sync.dma_start(si_t[:], si_d[:, :])

            W2 = sb.tile([128, K * 128], F32)
            nc.gpsimd.memset(W2[:], 0.0)
            for k in range(K):
                nc.sync.dma_start(W2[0:D, k * 128:k * 128 + D], W_d[k, :, :])
                nc.sync.dma_start(W2[D:128, k * 128 + D:(k + 1) * 128],
                                  W_d[k, :, :])

            hs_e = sb.tile([128, grp, D], F32)
            hs_o = sb.tile([128, grp, D], F32)
            nc.gpsimd.memset(hs_e[:], 0.0)
            nc.gpsimd.memset(hs_o[:], 0.0)

            def mid(gsrc, kk, cols, sout, scol):
                """gsrc[:, cols] (2-voxel 128-col chunks) -> transpose -> MM
                W2[kk] -> transpose back -> sout[:, scol:scol+128*len(cols)]."""
                gw = len(cols) * 128
                pa = ps.tile([128, 512], F32, tag="psA", space="PSUM")
                for j, cj in enumerate(cols):
                    nc.tensor.transpose(
                        out=pa[:, j * 128:(j + 1) * 128],
                        in_=gsrc[:, cj * 128:(cj + 1) * 128], identity=ident[:])
                ct = io.tile([128, 512], F32, tag="ct")
                nc.vector.tensor_copy(ct[:, :gw], pa[:, :gw])
                pb = ps.tile([128, 512], F32, tag="psB", space="PSUM")
                nc.tensor.matmul(out=pb[:, :gw],
                                 lhsT=W2[:, kk * 128:(kk + 1) * 128],
                                 rhs=ct[:, :gw], start=True, stop=True)
                hb = io.tile([128, 512], F32, tag="hb")
                nc.vector.tensor_copy(hb[:, :gw], pb[:, :gw])
                pc = ps.tile([128, 512], F32, tag="psC", space="PSUM")
                for j in range(len(cols)):
                    nc.tensor.transpose(
                        out=pc[:, j * 128:(j + 1) * 128],
                        in_=hb[:, j * 128:(j + 1) * 128], identity=ident[:])
                nc.scalar.activation(sout[:, scol:scol + gw], pc[:, :gw],
                                     mybir.ActivationFunctionType.Copy,
                                     bias=0.0)

            # ---- center tap: dense, strided-copied into h ----
            def center_group(g):
                rows = 512 if g == 24 else 1024
                nm = rows // 128
                xg = io.tile([128, 8, D], F32, tag="xg")
                xv = xc_d[g * 1024:g * 1024 + rows, :].rearrange(
                    "(m p) c -> p m c", p=128)
                nc.sync.dma_start(xg[:, 0:nm, :], xv)
                xgf = xg[:].rearrange("p m d -> p (m d)")
                cs = io.tile([128, 8, D], F32, tag="cs")
                csf = cs[:].rearrange("p m d -> p (m d)")
                mid(xgf, KC, list(range(nm // 2)), csf, 0)
                if g < 24:
                    nc.vector.tensor_copy(hs_e[:, 4 * g:4 * g + 4, :],
                                          cs[:, 0:8:2, :])
                    nc.vector.tensor_copy(hs_o[:, 4 * g:4 * g + 4, :],
                                          cs[:, 1:8:2, :])
                else:
                    nc.vector.tensor_copy(hs_e[:, 96:98, :], cs[:, 0:4:2, :])
                    nc.vector.tensor_copy(hs_o[:, 96:97, :], cs[:, 1:2, :])
                    nc.vector.tensor_copy(hs_o[0:40, 97:98, :],
                                          cs[0:40, 3:4, :])

            # ---- sparse taps (depth-2 software pipeline) ----
            def tap_gather(ki):
                k, NKk, koff = plan[ki]
                gb = io.tile([128, NKmax // 128, D], F32, tag="gb")
                nc.gpsimd.dma_gather(
                    out_ap=gb[:, 0:NKk // 128, :],
                    in_ap=xc_d[:, :],
                    idxs_ap=gi_t[:, koff // 16:(koff + NKk) // 16],
                    num_idxs=NKk, num_idxs_reg=NKk, elem_size=D,
                    single_packet=False, queue_num=ki % 4)
                return gb

            def tap_compute(ki, gb):
                k, NKk, koff = plan[ki]
                gbf = gb[:].rearrange("p m d -> p (m d)")
                sk = io.tile([128, NKmax // 128, D], F32, tag="sk")
                skf = sk[:].rearrange("p m d -> p (m d)")
                nch = NKk // 256
                for c0 in range(0, nch, 4):
                    cols = list(range(c0, min(c0 + 4, nch)))
                    mid(gbf, k, cols, skf, c0 * 128)
                nc.gpsimd.dma_scatter_add(
                    out_ap=hs_e[:], out_ap_other=hs_o[:],
                    in_ap=sk[:, 0:NKk // 128, :],
                    idxs_ap=si_t[:, koff // 16:(koff + NKk) // 16],
                    num_idxs=NKk, num_idxs_reg=NKk, elem_size=D,
                    single_packet=False, queue_num=ki % 4,
                    sbuf_tokens_per_rank=128, parity_reg=0)

            for g in range(25):
                center_group(g)

            pend = []
            for ki in range(len(plan)):
                pend.append((ki, tap_gather(ki)))
                if len(pend) > 2:
                    kj, gbj = pend.pop(0)
                    tap_compute(kj, gbj)
            for kj, gbj in pend:
                tap_compute(kj, gbj)

            # ---- BN stats from SBUF h ----
            scr = sb.tile([128, NSTAT * D], F32)
            s_pch = sb.tile([128, D], F32)
            q_pch = sb.tile([128, D], F32)
            s_tmp = sb.tile([128, D], F32)
            q_tmp = sb.tile([128, D], F32)
            for pi, hs in ((0, hs_e), (1, hs_o)):
                sview = hs[:, 0:NSTAT, :].rearrange("p g d -> p d g")
                so = s_pch if pi == 0 else s_tmp
                qo = q_pch if pi == 0 else q_tmp
                nc.vector.tensor_reduce(out=so[:], in_=sview,
                                        axis=mybir.AxisListType.X,
                                        op=mybir.AluOpType.add)
                hf = hs[:, 0:NSTAT, :].rearrange("p g d -> p (g d)")
                nc.vector.tensor_tensor(out=scr[:], in0=hf, in1=hf,
                                        op=mybir.AluOpType.mult)
                qview = scr[:].rearrange("p (g d) -> p d g", d=D)
                nc.vector.tensor_reduce(out=qo[:], in_=qview,
                                        axis=mybir.AxisListType.X,
                                        op=mybir.AluOpType.add)
            nc.vector.tensor_tensor(out=s_pch[:], in0=s_pch[:], in1=s_tmp[:],
                                    op=mybir.AluOpType.add)
            nc.vector.tensor_tensor(out=q_pch[:], in0=q_pch[:], in1=q_tmp[:],
                                    op=mybir.AluOpType.add)
            ones1 = sb.tile([128, 1], F32)
            nc.gpsimd.memset(ones1[:], 1.0)
            pS = ps.tile([64, 2], F32, tag="psS", space="PSUM")
            nc.tensor.matmul(out=pS[:, 0:1], lhsT=s_pch[:], rhs=ones1[:],
                             start=True, stop=True)
            nc.tensor.matmul(out=pS[:, 1:2], lhsT=q_pch[:], rhs=ones1[:],
                             start=True, stop=True)
            sq64 = sb.tile([64, 2], F32)
            nc.vector.tensor_copy(sq64[:], pS[:])

            cc_in = dram.tile([64, 2], F32)
            cc_out = dram.tile([64, 2], F32)
            nc.gpsimd.dma_start(cc_in[:], sq64[:])
            nc.gpsimd.collective_compute(
                "AllReduce", mybir.AluOpType.add,
                replica_groups=[list(range(C))],
                ins=[cc_in.opt()], outs=[cc_out.opt()])
            g2 = sb.tile([64, 2], F32)
            nc.sync.dma_start(g2[:], cc_out[:])

            # per-channel BN coefficients (channel-major on 64 partitions)
            me = sb.tile([64, 2], F32)
            nc.vector.tensor_scalar_mul(me[:], g2[:], 1.0 / N)  # [mean, Eh2]
            v1 = sb.tile([64, 1], F32)
            nc.vector.tensor_tensor(out=v1[:], in0=me[:, 0:1], in1=me[:, 0:1],
                                    op=mybir.AluOpType.mult)
            nc.vector.tensor_tensor(out=v1[:], in0=me[:, 1:2], in1=v1[:],
                                    op=mybir.AluOpType.subtract)
            eps_t = sb.tile([64, 1], F32)
            nc.gpsimd.memset(eps_t[:], EPS)
            std = sb.tile([64, 1], F32)
            nc.scalar.activation(std[:], v1[:],
                                 mybir.ActivationFunctionType.Sqrt,
                                 bias=eps_t[:])
            rin = sb.tile([64, 1], F32)
            nc.vector.reciprocal(rin[:], std[:])
            gam = sb.tile([64, 1], F32)
            nc.sync.dma_start(gam[:], gam_d[0, :, None])
            bet = sb.tile([64, 1], F32)
            nc.sync.dma_start(bet[:], bet_d[0, :, None])
            scb = sb.tile([64, 2], F32)
            nc.vector.tensor_tensor(out=scb[:, 0:1], in0=rin[:], in1=gam[:],
                                    op=mybir.AluOpType.mult)
            nc.vector.tensor_tensor(out=scb[:, 1:2], in0=me[:, 0:1],
                                    in1=scb[:, 0:1],
                                    op=mybir.AluOpType.mult)
            nc.vector.tensor_tensor(out=scb[:, 1:2], in0=bet[:],
                                    in1=scb[:, 1:2],
                                    op=mybir.AluOpType.subtract)

            # broadcast coefficients along partitions and free dim
            pT = ps.tile([128, 128], F32, tag="psS", space="PSUM")
            nc.tensor.transpose(out=pT[0:1, 0:64], in_=scb[:, 0:1],
                                identity=ident[0:64, 0:64])
            nc.tensor.transpose(out=pT[0:1, 64:128], in_=scb[:, 1:2],
                                identity=ident[0:64, 0:64])
            sr = sb.tile([1, 128], F32)
            nc.vector.tensor_copy(sr[:], pT[0:1, 0:128])
            onesrow = sb.tile([1, 128], F32)
            nc.gpsimd.memset(onesrow[:], 1.0)
            pB = ps.tile([128, 128], F32, tag="psB", space="PSUM")
            nc.tensor.matmul(out=pB[:, 0:64], lhsT=onesrow[:],
                             rhs=sr[:, 0:64], start=True, stop=True)
            nc.tensor.matmul(out=pB[:, 64:128], lhsT=onesrow[:],
                             rhs=sr[:, 64:128], start=True, stop=True)
            SB64 = sb.tile([128, 128], F32)
            nc.vector.tensor_copy(SB64[:], pB[:])
            S512 = sb.tile([128, 512], F32)
            B512 = sb.tile([128, 512], F32)
            for r in range(8):
                nc.vector.tensor_copy(S512[:, r * 64:(r + 1) * 64],
                                      SB64[:, 0:64])
                nc.vector.tensor_copy(B512[:, r * 64:(r + 1) * 64],
                                      SB64[:, 64:128])

            # ---- apply lrelu(h*s + b) in place, then write y ----
            CH = [(j * 512, min(512, NSTAT * D - j * 512))
                  for j in range((NSTAT * D + 511) // 512)]
            for hs in (hs_e, hs_o):
                hf = hs[:].rearrange("p g d -> p (g d)")
                for (o, ln) in CH:
                    t = io.tile([128, 512], F32, tag="ap")
                    nc.vector.tensor_tensor(out=t[:, 0:ln], in0=hf[:, o:o + ln],
                                            in1=S512[:, 0:ln],
                                            op=mybir.AluOpType.mult)
                    nc.vector.tensor_tensor(out=t[:, 0:ln], in0=t[:, 0:ln],
                                            in1=B512[:, 0:ln],
                                            op=mybir.AluOpType.add)
                    nc.scalar.activation(hf[:, o:o + ln], t[:, 0:ln],
                                         mybir.ActivationFunctionType.Lrelu,
                                         bias=0.0, scale=1.0, alpha=NEG)

            yv = y_d[0:24832, :].rearrange("(g two p) c -> two p g c",
                                           two=2, p=128)
            nc.sync.dma_start(yv[0], hs_e[:, 0:97, :])
            nc.sync.dma_start(yv[1], hs_o[:, 0:97, :])
            nc.sync.dma_start(y_d[24832:24960, :].rearrange("p c -> p c"),
                              hs_e[:, 97, :])
            nc.sync.dma_start(y_d[24960:25000, :].rearrange("p c -> p c"),
                              hs_o[0:40, 97, :])

    nc.compile()
    return nc


_CACHE = {}


def build(nbr):
    nbr = np.asarray(nbr)
    key = nbr.tobytes()[:4096] + nbr.tobytes()[-4096:]
    if key in _CACHE:
        return _CACHE[key]
    plan, GT, XL, grp, gslab, sslab, xsel, perm = _prep_host(
        np.asarray(nbr, np.int64))
    nc = _build_program(plan, GT, XL, grp)
    _CACHE[key] = (nc, gslab, sslab, xsel, perm, XL)
    return _CACHE[key]


def make_in_maps(x, W, gamma, beta, gslab, sslab, xsel, XL):
    x = np.ascontiguousarray(np.asarray(x, np.float32))
    W = np.ascontiguousarray(np.asarray(W, np.float32))
    gamma = np.asarray(gamma, np.float32).reshape(1, D)
    beta = np.asarray(beta, np.float32).reshape(1, D)
    in_maps = []
    for c in range(C):
        xc = np.zeros((XL, D), np.float32)
        xc[:len(xsel[c])] = x[xsel[c]]
        in_maps.append({
            "xc_d": xc,
            "W_d": W,
            "gam_d": gamma,
            "bet_d": beta,
            "gi_d": gslab[c],
            "si_d": sslab[c],
        })
    return in_maps


def kernel(x, W, gamma, beta, nbr):
    nc, gslab, sslab, xsel, perm, XL = build(nbr)
    in_maps = make_in_maps(x, W, gamma, beta, gslab, sslab, xsel, XL)
    res = bass_utils.run_bass_kernel_spmd(nc, in_maps, core_ids=list(range(C)))
    y_ranked = np.concatenate([res.results[c]["y_d"] for c in range(C)], axis=0)
    y = np.empty_like(y_ranked)
    y[perm] = y_ranked
    return y
